# revision 12
# baseline (speedup 1.0000x reference)
"""Distributed Trainium2 kernel for the sparse-attention layer.

Sharding: data-parallel over batch B=8 across the 8 NeuronCores (one batch
element per core).  The edge-list bias (attention_bias) is partitioned by its
batch column on the host and scattered into a dense per-batch [k, q] matrix
(transposed layout) that the device consumes with a diag(summed_keys) matmul.
No collectives are needed.

Device layout choice: everything is kept in "S^T" [k, q] layout so that the
softmax numerator P^T is produced directly in the layout the P@V matmul needs
(k on partitions) and the context comes out transposed [head*dh, q], which is
exactly the lhsT layout the output projection wants.  No P transposes.

  - statesT/keysT via PE transpose (identity matmul)
  - Q^T = Wq.T @ statesT, K^T = Wk.T @ keysT   [ha, n]
  - V = keysT.T @ Wv                           [n, ha]   (natural)
  - S^T_h[k,q] = K_h @ Q_h^T  (+ diag(sk_h) @ biasT  + causal NEG on the
    diagonal 128-blocks; upper triangle blocks skipped entirely)
  - P^T = exp(0.125 * S^T)  (no max subtraction needed: masked entries are
    -1e30 -> exp==0 exactly, logits are O(10) so no overflow)
  - ctx^T_h[a,q] += V_h^T @ P^T ; denominators collected into one [8, 512]
    PSUM tile via one-hot matmuls
  - ctx normalized with reciprocal(denoms) partition-broadcast, then
    out[t,d] = ctxn^T.T @ Wo
"""

import os
import numpy as np
from contextlib import ExitStack

import concourse.bass as bass
import concourse.mybir as mybir
import concourse.tile as tile
from concourse import bacc
from concourse.bass_utils import run_bass_kernel_spmd
from concourse.masks import make_identity

B, N, D, H, DH = 8, 512, 512, 8, 64
HD = H * DH  # 512
P = 128      # partitions
NT = N // P  # 4 tiles along n/k/q/t
DT = D // P  # 4 tiles along d
CT = HD // P  # 4 chunks along ha
NEG = -1.0e30
SCALE = 1.0 / 8.0  # 1/sqrt(DH)

F32 = mybir.dt.float32


def build_bass():
    nc = bacc.Bacc()

    d_states = nc.dram_tensor("states", [N, D], F32, kind="ExternalInput")
    d_keys = nc.dram_tensor("keys", [N, D], F32, kind="ExternalInput")
    d_biasT = nc.dram_tensor("biasT", [N, N], F32, kind="ExternalInput")
    d_sk = nc.dram_tensor("sk", [N, H], F32, kind="ExternalInput")
    d_wq = nc.dram_tensor("wq", [D, HD], F32, kind="ExternalInput")
    d_wk = nc.dram_tensor("wk", [D, HD], F32, kind="ExternalInput")
    d_wv = nc.dram_tensor("wv", [D, HD], F32, kind="ExternalInput")
    d_wo = nc.dram_tensor("wo", [HD, D], F32, kind="ExternalInput")
    d_out = nc.dram_tensor("out", [N, D], F32, kind="ExternalOutput")

    def tiled(dram, inner):
        # [R, C] dram -> [P, R//P, C] view (partition-major tiles)
        return dram[:, :].rearrange("(t p) c -> p t c", p=P)

    with ExitStack() as ctx:
        tc = ctx.enter_context(tile.TileContext(nc))
        consts = ctx.enter_context(tc.tile_pool(name="consts", bufs=1))
        big = ctx.enter_context(tc.tile_pool(name="big", bufs=1))
        ptp = ctx.enter_context(tc.tile_pool(name="ptp", bufs=4))
        outp = ctx.enter_context(tc.tile_pool(name="outp", bufs=2))
        psA = ctx.enter_context(tc.tile_pool(name="psA", bufs=2, space="PSUM"))
        psS = ctx.enter_context(tc.tile_pool(name="psS", bufs=2, space="PSUM"))
        psC = ctx.enter_context(tc.tile_pool(name="psC", bufs=2, space="PSUM"))
        psD = ctx.enter_context(tc.tile_pool(name="psD", bufs=2, space="PSUM"))

        # ---- constants -------------------------------------------------
        ident = consts.tile([P, P], F32)
        make_identity(nc, ident)
        # maskNEG[k, j] = NEG where (local q=j) < (local k), else 0
        maskneg = consts.tile([P, P], F32)
        nc.gpsimd.memset(maskneg, 0.0)
        nc.gpsimd.affine_select(
            out=maskneg,
            in_=maskneg,
            compare_op=mybir.AluOpType.is_ge,
            fill=NEG,
            base=0,
            pattern=[[1, P]],
            channel_multiplier=-1,
        )
        onehots = []
        for h in range(H):
            oh = consts.tile([P, H], F32, tag=f"onehot{h}", name=f"onehot{h}")
            nc.vector.memset(oh, 0.0)
            nc.vector.memset(oh[:, h : h + 1], 1.0)
            onehots.append(oh)

        # Absorb the gpsimd const-build deps into PE program order so the
        # first real transpose carries a single wait (S3_LW has one slot).
        warm_ps = psA.tile([P, P], F32, tag="psA")
        nc.tensor.transpose(warm_ps, ident, ident)
        nc.tensor.matmul(
            warm_ps[:, 0:H], lhsT=maskneg, rhs=onehots[0], start=True, stop=True
        )

        # ---- input DMAs ------------------------------------------------
        states_s = big.tile([P, NT, D], F32)
        keys_s = big.tile([P, NT, D], F32)
        biasT_s = big.tile([P, NT, N], F32)
        sk_s = big.tile([P, NT, H], F32)
        wq_s = big.tile([P, DT, HD], F32)
        wk_s = big.tile([P, DT, HD], F32)
        wv_s = big.tile([P, DT, HD], F32)
        wo_s = big.tile([P, CT, D], F32)
        nc.sync.dma_start(out=states_s, in_=tiled(d_states, D))
        nc.sync.dma_start(out=keys_s, in_=tiled(d_keys, D))
        nc.sync.dma_start(out=biasT_s, in_=tiled(d_biasT, N))
        nc.sync.dma_start(out=sk_s, in_=tiled(d_sk, H))
        nc.sync.dma_start(out=wq_s, in_=tiled(d_wq, HD))
        nc.sync.dma_start(out=wk_s, in_=tiled(d_wk, HD))
        nc.sync.dma_start(out=wv_s, in_=tiled(d_wv, HD))
        nc.sync.dma_start(out=wo_s, in_=tiled(d_wo, D))

        # ---- phase 1: transpose states/keys ---------------------------
        statesT_s = big.tile([P, DT, N], F32)  # [d, n]
        keysT_s = big.tile([P, DT, N], F32)
        for src, dst in ((states_s, statesT_s), (keys_s, keysT_s)):
            for dt_ in range(DT):
                ps = psA.tile([P, N], F32, tag="psA")
                for nt in range(NT):
                    nc.tensor.transpose(
                        ps[:, nt * P : (nt + 1) * P],
                        src[:, nt, dt_ * P : (dt_ + 1) * P],
                        ident,
                    )
                nc.any.tensor_copy(dst[:, dt_, :], ps)

        # ---- phase 2: projections -------------------------------------
        qt_s = big.tile([P, CT, N], F32)  # Q^T [ha, n]
        kt_s = big.tile([P, CT, N], F32)  # K^T [ha, n]
        v_s = big.tile([P, NT, HD], F32)  # V [n, ha]
        for w_s, x_s, o_s in ((wq_s, statesT_s, qt_s), (wk_s, keysT_s, kt_s)):
            for ct in range(CT):
                ps = psA.tile([P, N], F32, tag="psA")
                for dc in range(DT):
                    nc.tensor.matmul(
                        ps,
                        lhsT=w_s[:, dc, ct * P : (ct + 1) * P],
                        rhs=x_s[:, dc, :],
                        start=(dc == 0),
                        stop=(dc == DT - 1),
                    )
                nc.any.tensor_copy(o_s[:, ct, :], ps)
        for nt in range(NT):
            ps = psA.tile([P, HD], F32, tag="psA")
            for dc in range(DT):
                nc.tensor.matmul(
                    ps,
                    lhsT=keysT_s[:, dc, nt * P : (nt + 1) * P],
                    rhs=wv_s[:, dc, :],
                    start=(dc == 0),
                    stop=(dc == DT - 1),
                )
            nc.any.tensor_copy(v_s[:, nt, :], ps)

        # ---- phase 2.5: diag(sk) tiles --------------------------------
        # diag_s[p, kt, h, j] = (p == j) * sk[kt*128+p, h]
        diag_s = big.tile([P, NT, H, P], F32)
        for kt in range(NT):
            in0 = ident.unsqueeze(1).to_broadcast([P, H, P])
            in1 = sk_s[:, kt, :].unsqueeze(2).to_broadcast([P, H, P])
            nc.vector.tensor_mul(diag_s[:, kt, :, :], in0, in1)

        # ---- phase 3: attention ---------------------------------------
        ctxu_s = big.tile([P, CT, N], F32)  # unnormalized ctx^T [ha, q]
        den_ps = psD.tile([8, N], F32)  # row h = denominator of head h
        for c in range(H // 2):  # head pairs share a [128, N] psum bank
            ctx_ps = psC.tile([P, N], F32, tag="ctx")
            for hh in range(2):
                h = 2 * c + hh
                ht, po = h // 2, (h % 2) * DH
                for kt in range(NT):
                    q0 = kt * P  # first valid q column
                    nq = N - q0
                    s_ps = psS.tile([P, N], F32, tag="s")
                    # S^T = K_h @ Q_h^T
                    nc.tensor.matmul(
                        s_ps[:, q0:N],
                        lhsT=kt_s[po : po + DH, ht, kt * P : (kt + 1) * P],
                        rhs=qt_s[po : po + DH, ht, q0:N],
                        start=True,
                        stop=False,
                    )
                    # += diag(sk_h) @ biasT
                    nc.tensor.matmul(
                        s_ps[:, q0:N],
                        lhsT=diag_s[:, kt, h, :],
                        rhs=biasT_s[:, kt, q0:N],
                        start=False,
                        stop=False,
                    )
                    # += NEG on masked entries of the diagonal block
                    nc.tensor.matmul(
                        s_ps[:, q0 : q0 + P],
                        lhsT=ident,
                        rhs=maskneg,
                        start=False,
                        stop=True,
                        skip_group_check=True,
                    )
                    # P^T = exp(scale * S^T)
                    pt = ptp.tile([P, N], F32, tag="pt")
                    nc.scalar.activation(
                        out=pt[:, q0:N],
                        in_=s_ps[:, q0:N],
                        func=mybir.ActivationFunctionType.Exp,
                        scale=SCALE,
                    )
                    # ctx^T_h += V_h^T @ P^T
                    nc.tensor.matmul(
                        ctx_ps[po : po + DH, q0:N],
                        lhsT=v_s[:, kt, h * DH : (h + 1) * DH],
                        rhs=pt[:, q0:N],
                        start=(kt == 0),
                        stop=(kt == NT - 1),
                        skip_group_check=True,
                    )
                    # denominators: den[h, q] += sum_k P^T[k, q]
                    nc.tensor.matmul(
                        den_ps[:, q0:N],
                        lhsT=onehots[h],
                        rhs=pt[:, q0:N],
                        start=(h == 0 and kt == 0),
                        stop=(h == H - 1 and kt == NT - 1),
                        skip_group_check=True,
                    )

            # release the pair's psum bank promptly (normalize later)
            nc.any.tensor_copy(ctxu_s[:, c, :], ctx_ps)

        # ---- phase 4: normalize ---------------------------------------
        recip_s = big.tile([8, N], F32)
        nc.vector.reciprocal_approx_fast(out=recip_s, in_=den_ps)
        # broadcast recip rows across partitions via a DRAM bounce:
        # r_s[p, c, q] = recip[2c + p//64, q]
        d_recip = nc.dram_tensor("recip_scratch", [8, N], F32, kind="Internal")
        nc.sync.dma_start(out=d_recip[:, :], in_=recip_s)
        r_s = big.tile([P, CT, N], F32)
        base = d_recip[:, :]
        for p1 in range(2):
            src = bass.AP(
                tensor=base.tensor,
                offset=p1 * N,
                ap=[[0, DH], [2 * N, CT], [1, N]],
            )
            nc.sync.dma_start(out=r_s[p1 * DH : (p1 + 1) * DH, :, :], in_=src)
        ctxn_s = big.tile([P, CT, N], F32)  # normalized ctx^T [ha, q]
        for c in range(CT):
            nc.vector.tensor_mul(ctxn_s[:, c, :], ctxu_s[:, c, :], r_s[:, c, :])

        # ---- phase 5: output projection -------------------------------
        for t in range(NT):
            ps = psA.tile([P, D], F32, tag="psA")
            for cc in range(CT):
                nc.tensor.matmul(
                    ps,
                    lhsT=ctxn_s[:, cc, t * P : (t + 1) * P],
                    rhs=wo_s[:, cc, :],
                    start=(cc == 0),
                    stop=(cc == CT - 1),
                )
            o_t = outp.tile([P, D], F32, tag="ot")
            nc.any.tensor_copy(o_t, ps)
            nc.sync.dma_start(out=d_out[t * P : (t + 1) * P, :], in_=o_t)

    nc.compile()
    return nc


_NC = None


def _get_nc():
    global _NC
    if _NC is None:
        _NC = build_bass()
    return _NC


def _prep_in_maps(states, key_states, attention_bias, Wq, Wk, Wv, Wo,
                  bias_embs, bias_scalar):
    states = np.asarray(states, np.float32)
    key_states = np.asarray(key_states, np.float32)
    ab = np.asarray(attention_bias)
    et, b_idx, q_idx, k_idx = ab[:, 0], ab[:, 1], ab[:, 2], ab[:, 3]
    bias_vals = (np.asarray(bias_embs, np.float32)[et]
                 @ np.asarray(bias_scalar, np.float32))[:, 0]
    biasT = np.zeros((B, N, N), np.float32)
    np.add.at(biasT, (b_idx, k_idx, q_idx), bias_vals)
    # summed_keys[b,k,h] = sum_a (key @ Wk)[b,k,h,a] = key @ Wk.sum(-1)
    wk_sum = np.asarray(Wk, np.float32).reshape(D, H, DH).sum(-1)  # [D, H]
    sk = np.einsum("bnd,dh->bnh", key_states, wk_sum).astype(np.float32)
    wq = np.ascontiguousarray(np.asarray(Wq, np.float32).reshape(D, HD))
    wk = np.ascontiguousarray(np.asarray(Wk, np.float32).reshape(D, HD))
    wv = np.ascontiguousarray(np.asarray(Wv, np.float32).reshape(D, HD))
    wo = np.ascontiguousarray(np.asarray(Wo, np.float32).reshape(HD, D))
    in_maps = []
    for b in range(B):
        in_maps.append({
            "states": np.ascontiguousarray(states[b]),
            "keys": np.ascontiguousarray(key_states[b]),
            "biasT": np.ascontiguousarray(biasT[b]),
            "sk": np.ascontiguousarray(sk[b]),
            "wq": wq, "wk": wk, "wv": wv, "wo": wo,
        })
    return in_maps


def run(inputs, trace=False, tmpdir=None):
    """Returns (output [B,N,D] f32, BassKernelResults)."""
    nc = _get_nc()
    in_maps = _prep_in_maps(
        inputs["states"], inputs["key_states"], inputs["attention_bias"],
        inputs["Wq"], inputs["Wk"], inputs["Wv"], inputs["Wo"],
        inputs["bias_embs"], inputs["bias_scalar"],
    )
    res = run_bass_kernel_spmd(
        nc, in_maps, core_ids=list(range(B)), trace=trace, tmpdir=tmpdir
    )
    out = np.stack([res.results[b]["out"] for b in range(B)], axis=0)
    return out, res


def kernel(**inputs) -> np.ndarray:
    trace = bool(int(os.environ.get("BASS_KERNEL_TRACE", "0")))
    out, _ = run(inputs, trace=trace)
    return out
